# revision 1
# baseline (speedup 1.0000x reference)
"""IterNorm (iterative whitening normalization) Bass kernel for 8 TRN2 cores.

Reference (hardcoded shapes): X (64, 256, 56, 56) f32; g=4 groups of d=64
channels; m = 64*56*56 = 200704; Sigma = eps*I + (1/m) xc xc^T per group;
5 Newton-Schulz iters -> whitening wm; out = (wm @ xc) * weight + bias.

Sharding: data-parallel over batch B (8 b's per core). Per core:
  phase 1: stream local shard (natural layout [channels, hw]), accumulate
           per-group S = x x^T (PE transpose chunks + accumulating matmuls)
           and channel sums (DVE reduce). The first NRES of 16 (b, half)
           tiles stay resident in SBUF for phase 2.
  all-reduce 132KB of packed stats; replicated stats math + Newton-Schulz
           (2 groups packed per 128-tile via tile_position diag blocks).
  phase 2: apply out = W' @ x + offset (W' folds weight*sqrt(rTr)*P, offset
           folds bias - W' @ mean); resident tiles first, then reload rest.
"""

import numpy as np

B, C, H, W = 64, 256, 56, 56
HW = H * W               # 3136
G, D = 4, 64             # groups, channels/group
NCORES = 8
BS = B // NCORES         # 8 batches per core
M = B * HW               # 200704 (full reduction length)
EPS = 1e-5
T_ITERS = 5

NCH = 128                # transpose chunk width (hw)
FULL_CHUNKS = HW // NCH  # 24
TAIL = HW - FULL_CHUNKS * NCH  # 64
APPLY_N = 392            # apply matmul free dim; 8 * 392 = 3136
OUT_CHUNK = 1568         # output store chunk (2 per tile)
NRES = 10                # resident (b, half) tiles kept in SBUF

_CACHE = {}


def _build_nc(single_core_sim=False, repeat=1, bf16_stats=False, nres=NRES):
    import concourse.bacc as bacc
    import concourse.tile as tile
    from concourse import mybir

    f32 = mybir.dt.float32
    bf16 = mybir.dt.bfloat16
    st_dt = bf16 if bf16_stats else f32
    AX = mybir.AxisListType.X
    ADD = mybir.AluOpType.add
    SUB = mybir.AluOpType.subtract
    MULT = mybir.AluOpType.mult

    nc = bacc.Bacc(
        "TRN2",
        target_bir_lowering=False,
        debug=False,
        enable_asserts=False,
        num_devices=1 if single_core_sim else NCORES,
    )
    Xd = nc.dram_tensor("X", [BS, C, HW], f32, kind="ExternalInput").ap()
    Wd = nc.dram_tensor("weight", [C], f32, kind="ExternalInput").ap()
    Bd = nc.dram_tensor("bias", [C], f32, kind="ExternalInput").ap()
    Od = nc.dram_tensor("out", [BS, C, HW], f32, kind="ExternalOutput").ap()

    pairs = [(b, h) for b in range(BS) for h in range(2)]
    blksl = [slice(0, 64), slice(64, 128)]
    widths = [NCH] * FULL_CHUNKS + [TAIL]
    offs = [i * NCH for i in range(FULL_CHUNKS + 1)]
    blocks = [list(range(4 * kb, min(4 * kb + 4, 25))) for kb in range(7)]
    inv_m = 1.0 / float(M)

    with tile.TileContext(nc) as tc:
        with (
            tc.tile_pool(name="consts", bufs=1) as consts,
            tc.tile_pool(name="res", bufs=max(nres, 1)) as res,
            tc.tile_pool(name="p1x", bufs=2) as p1x,
            tc.tile_pool(name="p1t", bufs=4) as p1t,
            tc.tile_pool(name="statsp", bufs=2) as statsp,
            tc.tile_pool(name="nss", bufs=3) as nss,
            tc.tile_pool(name="apo", bufs=3) as apo,
            tc.tile_pool(name="dram", bufs=1, space="DRAM") as dram,
        ):
            # ---- constants (inline data, DMA'd once) ----
            id_np = np.eye(128, dtype=np.float32)
            gm_np = np.zeros((128, 2), dtype=np.float32)
            gm_np[0:64, 0] = 1.0
            gm_np[64:128, 1] = 1.0
            identity_d = nc.inline_tensor(id_np, name="identity_c")
            epsI_d = nc.inline_tensor(EPS * id_np, name="epsI_c")
            gmask_d = nc.inline_tensor(gm_np, name="gmask_c")
            ones_d = nc.inline_tensor(np.ones((1, 128), dtype=np.float32),
                                      name="ones_c")
            identity = consts.tile([128, 128], f32)
            nc.sync.dma_start(out=identity, in_=identity_d.ap())
            epsI = consts.tile([128, 128], f32)
            nc.sync.dma_start(out=epsI, in_=epsI_d.ap())
            gmask = consts.tile([128, 2], f32)
            nc.sync.dma_start(out=gmask, in_=gmask_d.ap())
            ones_row = consts.tile([1, 128], f32)
            nc.sync.dma_start(out=ones_row, in_=ones_d.ap())
            wrow = consts.tile([1, C], f32)
            nc.sync.dma_start(out=wrow, in_=Wd[None, :])
            bcol = consts.tile([128, 2], f32)
            nc.sync.dma_start(out=bcol[:, 0:1], in_=Bd[0:128][:, None])
            nc.sync.dma_start(out=bcol[:, 1:2], in_=Bd[128:256][:, None])

            for _rep in range(repeat):
                # ---- phase 1: local channel sums + covariance ----
                sums = statsp.tile([128, 2 * BS], f32, name="sums")
                cc_in = statsp.tile([128, 258], f32, name="cc_in")
                x_tiles = {}

                with tc.tile_pool(name="p1tp", bufs=4, space="PSUM") as p1tp, \
                     tc.tile_pool(name="covp", bufs=2, space="PSUM") as covp:
                    cov = [covp.tile([128, 128], f32, tag=f"cov{h}",
                                     name=f"cov{h}") for h in range(2)]
                    copy_eng = 0
                    for i, (b, h) in enumerate(pairs):
                        hs = slice(h * 128, (h + 1) * 128)
                        if i < nres:
                            xt = res.tile([128, HW], f32, tag="rxt", name="rxt")
                            x_tiles[(b, h)] = xt
                        else:
                            xt = p1x.tile([128, HW], f32, tag="xt", name="xt")
                        nc.sync.dma_start(out=xt, in_=Xd[b, hs, :])
                        nc.vector.reduce_sum(
                            out=sums[:, i:i + 1], in_=xt, axis=AX)
                        for kb, blk_chunks in enumerate(blocks):
                            pt = p1tp.tile([128, 512], f32, tag="pt", name="pt")
                            st = p1t.tile([128, 512], st_dt, tag="st", name="st")
                            for j, cidx in enumerate(blk_chunks):
                                kw = widths[cidx]
                                nc.tensor.transpose(
                                    pt[0:kw, j * NCH:j * NCH + 128],
                                    xt[:, offs[cidx]:offs[cidx] + kw],
                                    identity,
                                )
                            eng = nc.vector if copy_eng % 2 == 0 else nc.scalar
                            copy_eng += 1
                            kwall = 128 if len(blk_chunks) == 4 else widths[blk_chunks[0]]
                            fwall = len(blk_chunks) * NCH
                            if eng is nc.vector:
                                eng.tensor_copy(st[0:kwall, 0:fwall],
                                                pt[0:kwall, 0:fwall])
                            else:
                                eng.copy(st[0:kwall, 0:fwall],
                                         pt[0:kwall, 0:fwall])
                            for j, cidx in enumerate(blk_chunks):
                                kw = widths[cidx]
                                first = (i == 0) and (cidx == 0)
                                last = (i == len(pairs) - 1) and (cidx == 24)
                                nc.tensor.matmul(
                                    cov[h],
                                    st[0:kw, j * NCH:j * NCH + 128],
                                    st[0:kw, j * NCH:j * NCH + 128],
                                    start=first, stop=last,
                                )
                    nc.vector.tensor_copy(cc_in[:, 0:128], cov[0])
                    nc.vector.tensor_copy(cc_in[:, 128:256], cov[1])
                    nc.vector.reduce_sum(out=cc_in[:, 256:257],
                                         in_=sums[:, 0:16:2], axis=AX)
                    nc.vector.reduce_sum(out=cc_in[:, 257:258],
                                         in_=sums[:, 1:16:2], axis=AX)

                # ---- all-reduce ----
                bounce_in = dram.tile([128, 258], f32, tag="bin", name="bin")
                bounce_out = dram.tile([128, 258], f32, tag="bout", name="bout")
                nc.sync.dma_start(out=bounce_in, in_=cc_in)
                if single_core_sim:
                    nc.sync.dma_start(out=bounce_out, in_=bounce_in)
                else:
                    nc.gpsimd.collective_compute(
                        "AllReduce",
                        mybir.AluOpType.add,
                        replica_groups=[list(range(NCORES))],
                        ins=[bounce_in.opt()],
                        outs=[bounce_out.opt()],
                    )
                stats = statsp.tile([128, 258], f32, name="stats")
                nc.sync.dma_start(out=stats, in_=bounce_out)

                # ---- stats math + Newton-Schulz ----
                traces = statsp.tile([1, 4], f32, name="traces")
                mean_col = [statsp.tile([128, 1], f32, tag=f"mc{h}",
                                        name=f"mc{h}") for h in range(2)]
                Sig = [nss.tile([128, 128], f32, tag=f"sig{h}",
                                name=f"sig{h}") for h in range(2)]

                with tc.tile_pool(name="nsp", bufs=4, space="PSUM") as nsp:
                    for h in range(2):
                        nc.vector.tensor_scalar(
                            out=mean_col[h], in0=stats[:, 256 + h:257 + h],
                            scalar1=inv_m, scalar2=None, op0=MULT)
                        pmr = nsp.tile([128, 128], f32, tag="nsmisc", bufs=2,
                                       name="pmr")
                        nc.tensor.transpose(pmr[0:1, 0:128], mean_col[h],
                                            identity)
                        mrow = statsp.tile([1, 128], f32, tag=f"mr{h}",
                                           name=f"mr{h}")
                        nc.vector.tensor_copy(mrow, pmr[0:1, 0:128])
                        pouter = nsp.tile([128, 128], f32, tag="nsmisc", bufs=2,
                                          name="pouter")
                        nc.tensor.matmul(pouter, mrow, mrow, start=True,
                                         stop=True)
                        nc.vector.tensor_scalar(
                            out=Sig[h], in0=stats[:, h * 128:(h + 1) * 128],
                            scalar1=inv_m, scalar2=None, op0=MULT)
                        nc.vector.tensor_tensor(
                            out=Sig[h], in0=Sig[h], in1=pouter, op=SUB)
                        nc.vector.tensor_tensor(
                            out=Sig[h], in0=Sig[h], in1=epsI, op=ADD)
                        dtmp = nss.tile([128, 128], f32, tag="dtmp", name="dtmp")
                        nc.vector.tensor_tensor(out=dtmp, in0=Sig[h],
                                                in1=identity, op=MULT)
                        dcol = statsp.tile([128, 1], f32, tag=f"dc{h}",
                                           name=f"dc{h}")
                        nc.vector.reduce_sum(out=dcol, in_=dtmp, axis=AX)
                        ptr = nsp.tile([128, 128], f32, tag="nsmisc", bufs=2,
                                       name="ptr")
                        nc.tensor.matmul(ptr[0:1, 0:2], dcol, gmask,
                                         start=True, stop=True)
                        nc.vector.tensor_copy(traces[0:1, 2 * h:2 * h + 2],
                                              ptr[0:1, 0:2])

                    rtr = statsp.tile([1, 4], f32, name="rtr")
                    nc.vector.reciprocal(rtr, traces)
                    srtr = statsp.tile([1, 4], f32, name="srtr")
                    nc.scalar.sqrt(srtr, rtr)
                    pbc = nsp.tile([128, 128], f32, tag="nsmisc", bufs=2,
                                   name="pbc")
                    nc.tensor.matmul(pbc[:, 0:4], ones_row, rtr,
                                     start=True, stop=True)
                    nc.tensor.matmul(pbc[:, 4:8], ones_row, srtr,
                                     start=True, stop=True)
                    bc = statsp.tile([128, 8], f32, name="bc")
                    nc.vector.tensor_copy(bc, pbc[:, 0:8])
                    pwb = nsp.tile([128, 256], f32, tag="nsmisc", bufs=2,
                                   name="pwb")
                    nc.tensor.matmul(pwb, ones_row, wrow, start=True, stop=True)
                    wbc = nss.tile([128, 256], f32, tag="wbc", name="wbc")
                    nc.vector.tensor_copy(wbc, pwb)

                    rtr_col = [statsp.tile([128, 1], f32, tag=f"rc{h}",
                                           name=f"rc{h}") for h in range(2)]
                    srtr_col = [statsp.tile([128, 1], f32, tag=f"sc{h}",
                                            name=f"sc{h}") for h in range(2)]
                    wm = [nss.tile([128, 128], f32, tag=f"wm{h}",
                                   name=f"wm{h}") for h in range(2)]
                    offs_col = [statsp.tile([128, 1], f32, tag=f"of{h}",
                                            name=f"of{h}") for h in range(2)]

                    for h in range(2):
                        sel = statsp.tile([128, 2], f32, tag=f"sel{h}",
                                          name=f"sel{h}")
                        nc.vector.tensor_tensor(
                            out=sel, in0=bc[:, 2 * h:2 * h + 2], in1=gmask,
                            op=MULT)
                        nc.vector.reduce_sum(out=rtr_col[h], in_=sel, axis=AX)
                        sel2 = statsp.tile([128, 2], f32, tag=f"sel2{h}",
                                           name=f"sel2{h}")
                        nc.vector.tensor_tensor(
                            out=sel2, in0=bc[:, 4 + 2 * h:6 + 2 * h],
                            in1=gmask, op=MULT)
                        nc.vector.reduce_sum(out=srtr_col[h], in_=sel2, axis=AX)

                        sigN = nss.tile([128, 128], f32, tag=f"sn{h}",
                                        name=f"sn{h}")
                        nc.vector.tensor_scalar(
                            out=sigN, in0=Sig[h], scalar1=rtr_col[h],
                            scalar2=None, op0=MULT)

                        P = nss.tile([128, 128], f32, tag=f"P{h}", name=f"P{h}")
                        nc.vector.tensor_copy(P, identity)
                        for t in range(T_ITERS):
                            psA = nsp.tile([128, 128], f32, tag="nsmm", bufs=3,
                                           name="psA")
                            Asb = nss.tile([128, 128], f32, tag="Asb",
                                           name="Asb")
                            for k, sl in enumerate(blksl):
                                nc.tensor.matmul(
                                    psA[sl, sl], P[sl, sl], P[sl, sl],
                                    start=True, stop=True,
                                    tile_position=(64 * k, 64 * k))
                            for sl in blksl:
                                nc.vector.tensor_copy(Asb[sl, sl], psA[sl, sl])
                            psB = nsp.tile([128, 128], f32, tag="nsmm", bufs=3,
                                           name="psB")
                            Bsb = nss.tile([128, 128], f32, tag="Bsb",
                                           name="Bsb")
                            for k, sl in enumerate(blksl):
                                nc.tensor.matmul(
                                    psB[sl, sl], Asb[sl, sl], P[sl, sl],
                                    start=True, stop=True,
                                    tile_position=(64 * k, 64 * k))
                            for sl in blksl:
                                nc.vector.tensor_copy(Bsb[sl, sl], psB[sl, sl])
                            psC = nsp.tile([128, 128], f32, tag="nsmm", bufs=3,
                                           name="psC")
                            Csb = nss.tile([128, 128], f32, tag="Csb",
                                           name="Csb")
                            for k, sl in enumerate(blksl):
                                nc.tensor.matmul(
                                    psC[sl, sl], Bsb[sl, sl], sigN[sl, sl],
                                    start=True, stop=True,
                                    tile_position=(64 * k, 64 * k))
                            for sl in blksl:
                                nc.scalar.mul(Csb[sl, sl], psC[sl, sl], -0.5)
                            Pn = nss.tile([128, 128], f32, tag=f"P{h}",
                                          name=f"Pn{h}")
                            for sl in blksl:
                                nc.vector.tensor_scalar(
                                    out=Pn[sl, sl], in0=P[sl, sl],
                                    scalar1=1.5, scalar2=None, op0=MULT)
                                nc.vector.tensor_tensor(
                                    out=Pn[sl, sl], in0=Pn[sl, sl],
                                    in1=Csb[sl, sl], op=ADD)
                            P = Pn
                        for sl in blksl:
                            nc.vector.tensor_scalar(
                                out=wm[h][sl, sl], in0=P[sl, sl],
                                scalar1=srtr_col[h][sl, :], scalar2=None,
                                op0=MULT)
                            nc.vector.tensor_tensor(
                                out=wm[h][sl, sl], in0=wm[h][sl, sl],
                                in1=wbc[sl, h * 128 + sl.start:h * 128 + sl.stop],
                                op=MULT)
                        poff = nsp.tile([128, 128], f32, tag="nsmisc", bufs=2,
                                        name="poff")
                        for k, sl in enumerate(blksl):
                            nc.tensor.matmul(
                                poff[sl, 0:1], wm[h][sl, sl],
                                mean_col[h][sl, :],
                                start=True, stop=True,
                                tile_position=(64 * k, 64 * k))
                        nc.vector.tensor_tensor(
                            out=offs_col[h], in0=bcol[:, h:h + 1],
                            in1=poff[:, 0:1], op=SUB)

                # ---- phase 2: apply (resident tiles first, then reload) ----
                with tc.tile_pool(name="app", bufs=4, space="PSUM") as app:
                    for i, (b, h) in enumerate(pairs):
                        hs = slice(h * 128, (h + 1) * 128)
                        if i < nres:
                            xt = x_tiles[(b, h)]
                        else:
                            xt = p1x.tile([128, HW], f32, tag="xt", name="xt2")
                            nc.sync.dma_start(out=xt, in_=Xd[b, hs, :])
                        for oc in range(HW // OUT_CHUNK):
                            aot = apo.tile([128, OUT_CHUNK], f32, tag="aot",
                                           name="aot")
                            for k in range(OUT_CHUNK // APPLY_N):
                                gk = oc * (OUT_CHUNK // APPLY_N) + k
                                nsl = slice(gk * APPLY_N, (gk + 1) * APPLY_N)
                                osl = slice(k * APPLY_N, (k + 1) * APPLY_N)
                                pap = app.tile([128, APPLY_N], f32, tag="pap",
                                               name="pap")
                                for kk, sl in enumerate(blksl):
                                    nc.tensor.matmul(
                                        pap[sl, :], wm[h][sl, sl], xt[sl, nsl],
                                        start=True, stop=True,
                                        tile_position=(64 * kk, 64 * kk))
                                nc.vector.tensor_scalar(
                                    out=aot[:, osl], in0=pap,
                                    scalar1=offs_col[h], scalar2=None, op0=ADD)
                            nc.sync.dma_start(
                                out=Od[b, hs,
                                       oc * OUT_CHUNK:(oc + 1) * OUT_CHUNK],
                                in_=aot)
                if repeat > 1 and _rep < repeat - 1:
                    tc.strict_bb_all_engine_barrier()
    nc.compile()
    return nc


def kernel(X, weight, bias):
    from concourse.bass_utils import run_bass_kernel_spmd

    if "nc" not in _CACHE:
        _CACHE["nc"] = _build_nc()
    nc = _CACHE["nc"]

    X = np.ascontiguousarray(np.asarray(X, dtype=np.float32)).reshape(B, C, HW)
    w = np.ascontiguousarray(np.asarray(weight, dtype=np.float32)).reshape(C)
    bb = np.ascontiguousarray(np.asarray(bias, dtype=np.float32)).reshape(C)
    in_maps = [
        {"X": np.ascontiguousarray(X[i * BS:(i + 1) * BS]),
         "weight": w, "bias": bb}
        for i in range(NCORES)
    ]
    res = run_bass_kernel_spmd(nc, in_maps, core_ids=list(range(NCORES)))
    _CACHE["last_result"] = res
    out = np.concatenate([r["out"] for r in res.results], axis=0)
    return out.reshape(B, C, H, W)



# revision 13
# speedup vs baseline: 1.6027x; 1.6027x over previous
"""IterNorm (iterative whitening normalization) Bass kernel for 8 TRN2 cores.

Shapes (hardcoded): X (64, 256, 56, 56) f32; g=4 groups of d=64 channels;
m = 64*56*56 = 200704; Sigma = eps*I + (1/m) xc xc^T per group; 5
Newton-Schulz iters -> whitening wm; out = (wm @ xc) * weight + bias.

Sharding: data-parallel over batch B (8 b's per core). Per core:
  phase 1: cast-DMA the local shard f32->bf16 (all 16 (b,half) tiles stay
           resident in SBUF), PE-transpose 128-col chunks into PSUM tiles
           that carry persistent ones-columns, copy to bf16 st tiles, and
           accumulate per-half cov = st^T @ [st | ones] so the channel sums
           come out of the same matmuls (col 128 of the cov psum).
  all-reduce 66KB packed stats (4 diag 64x64 blocks + 2 sums columns).
  stats + Newton-Schulz: all 4 groups packed into one 128x128 "X layout"
           ([[g0, g2], [g1, g3]]), each product = 4 concurrent quadrant
           matmuls via tile_position; per iter A=P^2 and B=SigN*P computed
           in parallel, then C=A*B (halves the serial chain).
  phase 2: out = W'^T-style apply: lhsT_bd (block-diag, bf16, folds
           weight*sqrt(rTr)*P) @ x in 7 matmuls of N=448 per tile, offset
           (bias - W'@mean) added on alternating DVE/ACT while copying
           PSUM->SBUF staging, whole-tile 1.6MB output DMAs.
"""

import numpy as np

B, C, H, W = 64, 256, 56, 56
HW = H * W               # 3136
G, D = 4, 64             # groups, channels/group
NCORES = 8
BS = B // NCORES         # 8 batches per core
M = B * HW               # 200704 (full reduction length)
EPS = 1e-5
T_ITERS = 5

NCH = 128                # transpose chunk width (hw)
NCHUNK = 25              # 24 full chunks + 1 tail of 64
TAIL = HW - 24 * NCH     # 64
APPLY_N = 448            # apply matmul free dim; 7 * 448 = 3136

_CACHE = {}


def _build_nc(single_core_sim=False):
    import concourse.bacc as bacc
    import concourse.tile as tile
    from concourse import mybir
    import ml_dtypes

    f32 = mybir.dt.float32
    bf16 = mybir.dt.bfloat16
    AX = mybir.AxisListType.X
    ADD = mybir.AluOpType.add
    SUB = mybir.AluOpType.subtract
    MULT = mybir.AluOpType.mult

    nc = bacc.Bacc(
        "TRN2",
        target_bir_lowering=False,
        debug=False,
        enable_asserts=False,
        num_devices=1 if single_core_sim else NCORES,
    )
    Xd = nc.dram_tensor("X", [BS, C, HW], f32, kind="ExternalInput").ap()
    Wd = nc.dram_tensor("weight", [C], f32, kind="ExternalInput").ap()
    Bd = nc.dram_tensor("bias", [C], f32, kind="ExternalInput").ap()
    Od = nc.dram_tensor("out", [BS, C, HW], f32, kind="ExternalOutput").ap()

    widths = [NCH] * 24 + [TAIL]
    # 8 groups of 3 chunks + 1 tail group
    cgroups = [tuple(range(3 * i, 3 * i + 3)) for i in range(8)] + [(24,)]
    inv_m = 1.0 / float(M)
    blksl = [slice(0, 64), slice(64, 128)]

    id_np = np.eye(128, dtype=np.float32)
    eye2_np = np.vstack([np.eye(64, dtype=np.float32)] * 2)      # [128, 64]
    gmask_np = np.zeros((128, 2), dtype=np.float32)
    gmask_np[0:64, 0] = 1.0
    gmask_np[64:128, 1] = 1.0
    p0_np = np.kron(np.ones((2, 2), dtype=np.float32),
                    np.eye(64, dtype=np.float32))                # [128, 128]

    with tile.TileContext(nc) as tc:
        with (
            tc.tile_pool(name="consts", bufs=1) as consts,
            tc.tile_pool(name="resx", bufs=1) as resx,
            tc.tile_pool(name="stp", bufs=1) as stp,
            tc.tile_pool(name="statsp", bufs=2) as statsp,
            tc.tile_pool(name="nss", bufs=2) as nss,
            tc.tile_pool(name="apo", bufs=3) as apo,
            tc.tile_pool(name="dram", bufs=1, space="DRAM") as dram,
        ):
            # ---- constants ----
            id_bf = consts.tile([128, 128], bf16)
            nc.sync.dma_start(out=id_bf, in_=nc.inline_tensor(
                id_np.astype(ml_dtypes.bfloat16), name="id_bf").ap())
            id_f = consts.tile([128, 128], f32)
            nc.sync.dma_start(out=id_f, in_=nc.inline_tensor(
                id_np, name="id_f").ap())
            zeros_bf = consts.tile([128, 128], bf16)
            nc.sync.dma_start(out=zeros_bf, in_=nc.inline_tensor(
                np.zeros((128, 128), dtype=ml_dtypes.bfloat16),
                name="zeros_bf").ap())
            eye2 = consts.tile([128, 64], f32)
            nc.sync.dma_start(out=eye2, in_=nc.inline_tensor(
                eye2_np, name="eye2").ap())
            epsI2 = consts.tile([128, 64], f32)
            nc.sync.dma_start(out=epsI2, in_=nc.inline_tensor(
                EPS * eye2_np, name="epsI2").ap())
            gmask = consts.tile([128, 2], f32)
            nc.sync.dma_start(out=gmask, in_=nc.inline_tensor(
                gmask_np, name="gmask").ap())
            gmT2 = consts.tile([2, 128], f32)
            nc.sync.dma_start(out=gmT2, in_=nc.inline_tensor(
                np.ascontiguousarray(gmask_np.T), name="gmT2").ap())
            p0 = consts.tile([128, 128], f32)
            nc.sync.dma_start(out=p0, in_=nc.inline_tensor(
                p0_np, name="p0").ap())
            ones_row = consts.tile([1, 128], f32)
            nc.sync.dma_start(out=ones_row, in_=nc.inline_tensor(
                np.ones((1, 128), dtype=np.float32), name="ones_row").ap())
            ones11 = consts.tile([1, 1], f32)
            nc.sync.dma_start(out=ones11, in_=nc.inline_tensor(
                np.ones((1, 1), dtype=np.float32), name="ones11").ap())
            ones_col = consts.tile([128, 1], f32)
            nc.sync.dma_start(out=ones_col, in_=nc.inline_tensor(
                np.ones((128, 1), dtype=np.float32), name="ones_col").ap())
            wrow = consts.tile([1, C], f32)
            nc.sync.dma_start(out=wrow, in_=Wd[None, :])
            bcol = consts.tile([128, 2], f32)
            nc.sync.dma_start(out=bcol[:, 0:1], in_=Bd[0:128][:, None])
            nc.sync.dma_start(out=bcol[:, 1:2], in_=Bd[128:256][:, None])

            # ---- resident input tiles: cast-DMA f32 -> bf16 ----
            xt = []
            for b in range(BS):
                t = resx.tile([128, 2 * HW], bf16, name=f"xt{b}")
                nc.gpsimd.dma_start(out=t[:, 0:HW], in_=Xd[b, 0:128, :])
                nc.gpsimd.dma_start(out=t[:, HW:2 * HW], in_=Xd[b, 128:256, :])
                xt.append(t)

            # persistent bf16 transposed-chunk tiles (ones col interleaved)
            st_bufs = [stp.tile([128, 390], bf16, name=f"stb{i}")
                       for i in range(4)]

            cc_in = statsp.tile([128, 130], f32, name="cc_in")

            # ---- phase 1: transposes + cov/sums accumulation ----
            with tc.tile_pool(name="p1ps", bufs=1, space="PSUM") as p1ps:
                pt_bufs = [p1ps.tile([128, 512], bf16, name=f"ptb{i}")
                           for i in range(2)]
                cov = [p1ps.tile([128, 512], f32, name=f"cov{h}")
                       for h in range(2)]
                for ptb in pt_bufs:
                    for oc in (128, 258, 388):
                        nc.vector.memset(
                            ptb[:, oc:oc + 2].bitcast(mybir.dt.uint32),
                            0x3F803F80)

                gctr = 0
                for b in range(BS):
                    for h in range(2):
                        base = h * HW
                        for chunks in cgroups:
                            ptb = pt_bufs[gctr % 2]
                            stb = st_bufs[gctr % 4]
                            ncols = len(chunks) * 130
                            for j, k in enumerate(chunks):
                                kw = widths[k]
                                nc.tensor.transpose(
                                    ptb[0:kw, j * 130:j * 130 + 128],
                                    xt[b][:, base + k * NCH:
                                          base + k * NCH + kw],
                                    id_bf,
                                )
                            # ACT faults on bf16 PSUM reads; DVE only
                            nc.vector.tensor_copy(
                                stb[0:128, 0:ncols], ptb[0:128, 0:ncols])
                            for j, k in enumerate(chunks):
                                kw = widths[k]
                                nc.tensor.matmul(
                                    cov[h][:, 0:129],
                                    stb[0:kw, j * 130:j * 130 + 128],
                                    stb[0:kw, j * 130:j * 130 + 129],
                                    start=(b == 0 and k == 0),
                                    stop=(b == BS - 1 and k == NCHUNK - 1),
                                )
                            gctr += 1

                # pack AR payload: X-layout diag blocks + sums columns
                nc.vector.tensor_copy(cc_in[0:64, 0:64], cov[0][0:64, 0:64])
                nc.scalar.copy(cc_in[64:128, 0:64], cov[0][64:128, 64:128])
                nc.vector.tensor_copy(cc_in[0:64, 64:128], cov[1][0:64, 0:64])
                nc.scalar.copy(cc_in[64:128, 64:128],
                               cov[1][64:128, 64:128])
                nc.vector.tensor_copy(cc_in[:, 128:129], cov[0][:, 128:129])
                nc.scalar.copy(cc_in[:, 129:130], cov[1][:, 128:129])

            # ---- all-reduce ----
            bounce_in = dram.tile([128, 130], f32, name="bin")
            bounce_out = dram.tile([128, 130], f32, name="bout")
            nc.sync.dma_start(out=bounce_in, in_=cc_in)
            if single_core_sim:
                nc.sync.dma_start(out=bounce_out, in_=bounce_in)
            else:
                nc.gpsimd.collective_compute(
                    "AllReduce",
                    mybir.AluOpType.add,
                    replica_groups=[list(range(NCORES))],
                    ins=[bounce_in.opt()],
                    outs=[bounce_out.opt()],
                )
            stats = statsp.tile([128, 130], f32, name="stats")
            nc.sync.dma_start(out=stats, in_=bounce_out)

            # ---- stats math + Newton-Schulz (per-half, diag-packed) ----
            bd = [nss.tile([128, 128], bf16, name=f"bd{h}") for h in range(2)]
            offs = [statsp.tile([128, 1], f32, name=f"offs{h}")
                    for h in range(2)]

            with tc.tile_pool(name="nsps", bufs=1, space="PSUM") as nsps:
                # consolidated PSUM scratch (bank-quantized: 4 banks total)
                scrA = nsps.tile([128, 512], f32, name="scrA")
                po4 = nsps.tile([128, 512], f32, name="po4")
                nsh = [nsps.tile([128, 512], f32, name=f"nsh{h}")
                       for h in range(2)]

                # means
                mean2 = statsp.tile([128, 2], f32, name="mean2")
                nc.vector.tensor_scalar(out=mean2, in0=stats[:, 128:130],
                                        scalar1=inv_m, scalar2=None, op0=MULT)
                mrow = []
                for h in range(2):
                    pmr = scrA[0:1, 128 * h:128 * h + 128]
                    nc.tensor.transpose(pmr, mean2[:, h:h + 1], id_f)
                    mr = statsp.tile([1, 128], f32, name=f"mrow{h}")
                    nc.vector.tensor_copy(mr, pmr)
                    mrow.append(mr)
                pouter = [po4[:, 0:128], po4[:, 128:256]]
                for h in range(2):
                    nc.tensor.matmul(pouter[h], mrow[h], mrow[h],
                                     start=True, stop=True)

                # Sigma per half in packed columns [g_top; g_bot] [128, 64]
                sig = nss.tile([128, 128], f32, name="sig")
                tcol = []
                for h in range(2):
                    csl = slice(64 * h, 64 * h + 64)
                    nc.vector.tensor_scalar(
                        out=sig[:, csl], in0=stats[:, csl],
                        scalar1=inv_m, scalar2=None, op0=MULT)
                    for k, sl in enumerate(blksl):
                        nc.vector.tensor_tensor(
                            out=sig[sl, csl], in0=sig[sl, csl],
                            in1=pouter[h][sl, sl], op=SUB)
                    nc.vector.tensor_tensor(
                        out=sig[:, csl], in0=sig[:, csl], in1=epsI2, op=ADD)
                    # diag -> per-partition column -> per-group traces
                    dtmp = nss.tile([128, 64], f32, tag="dtmp", name="dtmp")
                    nc.vector.tensor_tensor(out=dtmp, in0=sig[:, csl],
                                            in1=eye2, op=MULT)
                    tc_h = statsp.tile([128, 1], f32, name=f"tcol{h}")
                    nc.vector.reduce_sum(out=tc_h, in_=dtmp, axis=AX)
                    tcol.append(tc_h)

                for h in range(2):
                    nc.tensor.matmul(scrA[0:1, 256 + 2 * h:258 + 2 * h],
                                     tcol[h], gmask, start=True, stop=True)
                traces = statsp.tile([1, 4], f32, name="traces")
                nc.vector.tensor_copy(traces, scrA[0:1, 256:260])
                rtr = statsp.tile([1, 4], f32, name="rtr")
                nc.vector.reciprocal(rtr, traces)
                srtr = statsp.tile([1, 4], f32, name="srtr")
                nc.scalar.sqrt(srtr, rtr)

                # broadcast rtr/srtr down partitions, select per group
                nc.tensor.matmul(scrA[:, 264:268], ones_row, rtr,
                                 start=True, stop=True)
                nc.tensor.matmul(scrA[:, 268:272], ones_row, srtr,
                                 start=True, stop=True)
                bc = statsp.tile([128, 8], f32, name="bc")
                nc.vector.tensor_copy(bc, scrA[:, 264:272])
                # weight broadcast [128, 256]
                nc.tensor.matmul(po4[:, 256:512], ones_row, wrow,
                                 start=True, stop=True)
                wbc = nss.tile([128, 256], f32, name="wbc")
                nc.vector.tensor_copy(wbc, po4[:, 256:512])

                rcol, scol = [], []
                for h in range(2):
                    sel = statsp.tile([128, 2], f32, tag="sel", name="sel")
                    nc.vector.tensor_tensor(out=sel, in0=bc[:, 2 * h:2 * h + 2],
                                            in1=gmask, op=MULT)
                    rc = statsp.tile([128, 1], f32, name=f"rcol{h}")
                    nc.vector.reduce_sum(out=rc, in_=sel, axis=AX)
                    rcol.append(rc)
                    sel2 = statsp.tile([128, 2], f32, tag="sel", name="sel2")
                    nc.vector.tensor_tensor(
                        out=sel2, in0=bc[:, 4 + 2 * h:6 + 2 * h],
                        in1=gmask, op=MULT)
                    sc = statsp.tile([128, 1], f32, name=f"scol{h}")
                    nc.vector.reduce_sum(out=sc, in_=sel2, axis=AX)
                    scol.append(sc)

                # SigN block-diag per half
                sigN = []
                for h in range(2):
                    csl = slice(64 * h, 64 * h + 64)
                    sn = nss.tile([128, 128], f32, name=f"sigN{h}")
                    for k, sl in enumerate(blksl):
                        nc.vector.tensor_scalar(
                            out=sn[sl, sl], in0=sig[sl, csl],
                            scalar1=rcol[h][sl, :], scalar2=None, op0=MULT)
                    sigN.append(sn)

                # Newton-Schulz, 5 iters, both halves interleaved;
                # per iter: A = P*P and B = SigN*P in parallel, C = A*B
                P = [id_f, id_f]
                for t in range(T_ITERS):
                    pn, asb, bsb, cs = [], [], [], []
                    for h in range(2):
                        p_ = nss.tile([128, 128], f32, tag=f"P{h}",
                                      name=f"P{h}_{t}")
                        for sl in blksl:
                            nc.vector.tensor_scalar(
                                out=p_[sl, sl], in0=P[h][sl, sl],
                                scalar1=1.5, scalar2=None, op0=MULT)
                        pn.append(p_)
                    for h in range(2):
                        for k, sl in enumerate(blksl):
                            nc.tensor.matmul(
                                nsh[h][sl, 0:64], P[h][sl, sl], P[h][sl, sl],
                                start=True, stop=True,
                                tile_position=(64 * k, 64 * k))
                        for k, sl in enumerate(blksl):
                            nc.tensor.matmul(
                                nsh[h][sl, 128:192], sigN[h][sl, sl],
                                P[h][sl, sl], start=True, stop=True,
                                tile_position=(64 * k, 64 * k))
                    for h in range(2):
                        a_ = nss.tile([128, 64], f32, tag=f"asb{h}",
                                      name="asb")
                        nc.vector.tensor_copy(a_, nsh[h][:, 0:64])
                        asb.append(a_)
                        b_ = nss.tile([128, 64], f32, tag=f"bsb{h}",
                                      name="bsb")
                        nc.scalar.copy(b_, nsh[h][:, 128:192])
                        bsb.append(b_)
                    for h in range(2):
                        for k, sl in enumerate(blksl):
                            nc.tensor.matmul(
                                nsh[h][sl, 256:320], asb[h][sl, :],
                                bsb[h][sl, :], start=True, stop=True,
                                tile_position=(64 * k, 64 * k))
                    for h in range(2):
                        c_ = nss.tile([128, 64], f32, tag=f"cs{h}", name="cs")
                        nc.scalar.mul(c_, nsh[h][:, 256:320], -0.5)
                        for k, sl in enumerate(blksl):
                            nc.vector.tensor_tensor(
                                out=pn[h][sl, sl], in0=pn[h][sl, sl],
                                in1=c_[sl, :], op=ADD)
                    P = pn

                # extraction: block-diag bf16 lhsT per half + offsets
                for h in range(2):
                    nc.vector.tensor_copy(bd[h], zeros_bf)
                    for k, sl in enumerate(blksl):
                        nc.vector.tensor_scalar(
                            out=bd[h][sl, sl], in0=P[h][sl, sl],
                            scalar1=scol[h][sl, :], scalar2=None, op0=MULT)
                        nc.vector.tensor_tensor(
                            out=bd[h][sl, sl], in0=bd[h][sl, sl],
                            in1=wbc[sl, 128 * h + sl.start:
                                    128 * h + sl.stop], op=MULT)
                    mb = statsp.tile([128, 1], bf16, name=f"meanbf{h}")
                    nc.vector.tensor_copy(mb, mean2[:, h:h + 1])
                    pwm = scrA[:, 280 + h:281 + h]
                    nc.tensor.matmul(pwm, bd[h], mb, start=True, stop=True)
                    nc.vector.tensor_tensor(
                        out=offs[h], in0=bcol[:, h:h + 1],
                        in1=pwm, op=SUB)

            # ---- phase 2: apply ----
            with tc.tile_pool(name="app", bufs=4, space="PSUM") as app:
                for h in range(2):
                    hs = slice(h * 128, (h + 1) * 128)
                    base = h * HW
                    for b in range(BS):
                        aot = apo.tile([128, HW], f32, tag="aot", name="aot")
                        for ci in range(HW // APPLY_N):
                            nsl = slice(base + ci * APPLY_N,
                                        base + (ci + 1) * APPLY_N)
                            osl = slice(ci * APPLY_N, (ci + 1) * APPLY_N)
                            pap = app.tile([128, APPLY_N], f32, tag="pap",
                                           name="pap")
                            nc.tensor.matmul(pap, bd[h], xt[b][:, nsl],
                                             start=True, stop=True)
                            if (ci + b) % 2 == 0:
                                nc.vector.tensor_scalar(
                                    out=aot[:, osl], in0=pap,
                                    scalar1=offs[h], scalar2=None, op0=ADD)
                            else:
                                nc.scalar.add(aot[:, osl], pap, offs[h])
                        nc.sync.dma_start(out=Od[b, hs, :], in_=aot)
    nc.compile()
    return nc


def kernel(X, weight, bias):
    from concourse.bass_utils import run_bass_kernel_spmd

    if "nc" not in _CACHE:
        _CACHE["nc"] = _build_nc()
    nc = _CACHE["nc"]

    X = np.ascontiguousarray(np.asarray(X, dtype=np.float32)).reshape(B, C, HW)
    w = np.ascontiguousarray(np.asarray(weight, dtype=np.float32)).reshape(C)
    bb = np.ascontiguousarray(np.asarray(bias, dtype=np.float32)).reshape(C)
    in_maps = [
        {"X": np.ascontiguousarray(X[i * BS:(i + 1) * BS]),
         "weight": w, "bias": bb}
        for i in range(NCORES)
    ]
    res = run_bass_kernel_spmd(nc, in_maps, core_ids=list(range(NCORES)))
    _CACHE["last_result"] = res
    out = np.concatenate([r["out"] for r in res.results], axis=0)
    return out.reshape(B, C, H, W)


# revision 14
# speedup vs baseline: 1.6814x; 1.0491x over previous
"""IterNorm (iterative whitening normalization) Bass kernel for 8 TRN2 cores.

Shapes (hardcoded): X (64, 256, 56, 56) f32; g=4 groups of d=64 channels;
m = 64*56*56 = 200704; Sigma = eps*I + (1/m) xc xc^T per group; 5
Newton-Schulz iters -> whitening wm; out = (wm @ xc) * weight + bias.

Sharding: data-parallel over batch B (8 b's per core), with the channel
halves (h = channels 0:128 / 128:256, i.e. groups {0,1} / {2,3}) pipelined
so the two 33KB stats all-reduces hide under compute:

  [h0 phase-1: transpose+cov 8 tiles] -> AR(h0) in flight
  [h1 phase-1]                           (covers AR(h0) latency)
  [h0 stats+NewtonSchulz]  -> AR(h1) in flight right after h1 cov stop
  [h0 apply + output DMA]                (covers AR(h1) latency)
  [h1 stats+NS] [h1 apply + output DMA]

Phase 1 tricks: cast-DMA f32->bf16 loads (all tiles resident in SBUF); PE
transposes into bf16 PSUM tiles with persistent ones-columns; cov matmuls
use rhs [chunk | ones] so channel sums fall out of col 128 for free; DVE
does the PSUM->SBUF copies (ACT faults on bf16 PSUM reads).
Newton-Schulz packs each half's 2 groups into diagonal 64x64 quadrants
(tile_position), computing A=P^2 and B=SigN*P back-to-back then C=A*B to
halve the serial chain. Apply is a single block-diagonal bf16 128-contract
matmul per 448-col chunk (weight*sqrt(rTr)*P folded into lhsT, bias-W@mean
folded into a per-partition offset added on alternating DVE/ACT).
"""

import numpy as np

B, C, H, W = 64, 256, 56, 56
HW = H * W               # 3136
G, D = 4, 64             # groups, channels/group
NCORES = 8
BS = B // NCORES         # 8 batches per core
M = B * HW               # 200704 (full reduction length)
EPS = 1e-5
T_ITERS = 5

NCH = 128                # transpose chunk width (hw)
NCHUNK = 25              # 24 full chunks + 1 tail of 64
TAIL = HW - 24 * NCH     # 64
APPLY_N = 448            # apply matmul free dim; 7 * 448 = 3136

_CACHE = {}


def _build_nc(single_core_sim=False):
    import concourse.bacc as bacc
    import concourse.tile as tile
    from concourse import mybir
    import ml_dtypes

    f32 = mybir.dt.float32
    bf16 = mybir.dt.bfloat16
    AX = mybir.AxisListType.X
    ADD = mybir.AluOpType.add
    SUB = mybir.AluOpType.subtract
    MULT = mybir.AluOpType.mult

    nc = bacc.Bacc(
        "TRN2",
        target_bir_lowering=False,
        debug=False,
        enable_asserts=False,
        num_devices=1 if single_core_sim else NCORES,
    )
    Xd = nc.dram_tensor("X", [BS, C, HW], f32, kind="ExternalInput").ap()
    Wd = nc.dram_tensor("weight", [C], f32, kind="ExternalInput").ap()
    Bd = nc.dram_tensor("bias", [C], f32, kind="ExternalInput").ap()
    Od = nc.dram_tensor("out", [BS, C, HW], f32, kind="ExternalOutput").ap()

    widths = [NCH] * 24 + [TAIL]
    cgroups = [tuple(range(3 * i, 3 * i + 3)) for i in range(8)] + [(24,)]
    inv_m = 1.0 / float(M)
    blksl = [slice(0, 64), slice(64, 128)]

    id_np = np.eye(128, dtype=np.float32)
    eye2_np = np.vstack([np.eye(64, dtype=np.float32)] * 2)      # [128, 64]
    gmask_np = np.zeros((128, 2), dtype=np.float32)
    gmask_np[0:64, 0] = 1.0
    gmask_np[64:128, 1] = 1.0

    with tile.TileContext(nc) as tc:
        with (
            tc.tile_pool(name="consts", bufs=1) as consts,
            tc.tile_pool(name="resx", bufs=1) as resx,
            tc.tile_pool(name="stp", bufs=1) as stp,
            tc.tile_pool(name="statsp", bufs=1) as statsp,
            tc.tile_pool(name="nss", bufs=2) as nss,
            tc.tile_pool(name="apo", bufs=3) as apo,
            tc.tile_pool(name="dram", bufs=1, space="DRAM") as dram,
        ):
            # ---- constants ----
            id_bf = consts.tile([128, 128], bf16)
            nc.sync.dma_start(out=id_bf, in_=nc.inline_tensor(
                id_np.astype(ml_dtypes.bfloat16), name="id_bf").ap())
            id_f = consts.tile([128, 128], f32)
            nc.sync.dma_start(out=id_f, in_=nc.inline_tensor(
                id_np, name="id_f").ap())
            zeros_bf = consts.tile([128, 128], bf16)
            nc.sync.dma_start(out=zeros_bf, in_=nc.inline_tensor(
                np.zeros((128, 128), dtype=ml_dtypes.bfloat16),
                name="zeros_bf").ap())
            eye2 = consts.tile([128, 64], f32)
            nc.sync.dma_start(out=eye2, in_=nc.inline_tensor(
                eye2_np, name="eye2").ap())
            epsI2 = consts.tile([128, 64], f32)
            nc.sync.dma_start(out=epsI2, in_=nc.inline_tensor(
                EPS * eye2_np, name="epsI2").ap())
            gmask = consts.tile([128, 2], f32)
            nc.sync.dma_start(out=gmask, in_=nc.inline_tensor(
                gmask_np, name="gmask").ap())
            ones_row = consts.tile([1, 128], f32)
            nc.sync.dma_start(out=ones_row, in_=nc.inline_tensor(
                np.ones((1, 128), dtype=np.float32), name="ones_row").ap())
            wrow = consts.tile([1, C], f32)
            nc.sync.dma_start(out=wrow, in_=Wd[None, :])
            bcol = consts.tile([128, 2], f32)
            nc.sync.dma_start(out=bcol[:, 0:1], in_=Bd[0:128][:, None])
            nc.sync.dma_start(out=bcol[:, 1:2], in_=Bd[128:256][:, None])

            # ---- resident input tiles: cast-DMA f32 -> bf16, h-major ----
            xt = [resx.tile([128, 2 * HW], bf16, name=f"xt{b}")
                  for b in range(BS)]
            for h in range(2):
                for b in range(BS):
                    nc.gpsimd.dma_start(
                        out=xt[b][:, h * HW:(h + 1) * HW],
                        in_=Xd[b, 128 * h:128 * (h + 1), :])

            st_bufs = [stp.tile([128, 390], bf16, name=f"stb{i}")
                       for i in range(4)]
            cc_in = [statsp.tile([128, 66], f32, name=f"cc_in{h}")
                     for h in range(2)]
            stats = [statsp.tile([128, 66], f32, name=f"stats{h}")
                     for h in range(2)]
            bounce_in = [dram.tile([128, 66], f32, name=f"bin{h}")
                         for h in range(2)]
            bounce_out = [dram.tile([128, 66], f32, name=f"bout{h}")
                          for h in range(2)]
            bd = [nss.tile([128, 128], bf16, name=f"bd{h}") for h in range(2)]
            offs = [statsp.tile([128, 1], f32, name=f"offs{h}")
                    for h in range(2)]

            with (
                tc.tile_pool(name="p1ps", bufs=1, space="PSUM") as p1ps,
                tc.tile_pool(name="nsps", bufs=1, space="PSUM") as nsps,
                tc.tile_pool(name="app", bufs=2, space="PSUM") as app,
            ):
                pt_bufs = [p1ps.tile([128, 512], bf16, name=f"ptb{i}")
                           for i in range(2)]
                cov = [p1ps.tile([128, 512], f32, name=f"cov{h}")
                       for h in range(2)]
                scrA = nsps.tile([128, 512], f32, name="scrA")
                nsh01 = nsps.tile([128, 512], f32, name="nsh01")

                for ptb in pt_bufs:
                    for oc in (128, 258, 388):
                        nc.vector.memset(
                            ptb[:, oc:oc + 2].bitcast(mybir.dt.uint32),
                            0x3F803F80)

                # weight broadcast [128, 256] (h-independent, done once)
                nc.tensor.matmul(scrA[:, 0:256], ones_row, wrow,
                                 start=True, stop=True)
                wbc = nss.tile([128, 256], f32, name="wbc")
                nc.vector.tensor_copy(wbc, scrA[:, 0:256])

                gctr = 0

                def phase1_half(h):
                    nonlocal gctr
                    base = h * HW
                    for b in range(BS):
                        for chunks in cgroups:
                            ptb = pt_bufs[gctr % 2]
                            stb = st_bufs[gctr % 4]
                            ncols = len(chunks) * 130
                            for j, k in enumerate(chunks):
                                kw = widths[k]
                                nc.tensor.transpose(
                                    ptb[0:kw, j * 130:j * 130 + 128],
                                    xt[b][:, base + k * NCH:
                                          base + k * NCH + kw],
                                    id_bf,
                                )
                            # ACT faults on bf16 PSUM reads; DVE-only copy
                            nc.vector.tensor_copy(
                                stb[0:128, 0:ncols], ptb[0:128, 0:ncols])
                            for j, k in enumerate(chunks):
                                kw = widths[k]
                                nc.tensor.matmul(
                                    cov[h][:, 0:129],
                                    stb[0:kw, j * 130:j * 130 + 128],
                                    stb[0:kw, j * 130:j * 130 + 129],
                                    start=(b == 0 and k == 0),
                                    stop=(b == BS - 1 and k == NCHUNK - 1),
                                )
                            gctr += 1

                def start_allreduce(h):
                    nc.vector.tensor_copy(cc_in[h][0:64, 0:64],
                                          cov[h][0:64, 0:64])
                    nc.vector.tensor_copy(cc_in[h][64:128, 0:64],
                                          cov[h][64:128, 64:128])
                    nc.vector.tensor_copy(cc_in[h][:, 64:65],
                                          cov[h][:, 128:129])
                    nc.sync.dma_start(out=bounce_in[h], in_=cc_in[h])
                    if single_core_sim:
                        nc.sync.dma_start(out=bounce_out[h],
                                          in_=bounce_in[h])
                    else:
                        nc.gpsimd.collective_compute(
                            "AllReduce",
                            mybir.AluOpType.add,
                            replica_groups=[list(range(NCORES))],
                            ins=[bounce_in[h].opt()],
                            outs=[bounce_out[h].opt()],
                        )
                    nc.sync.dma_start(out=stats[h], in_=bounce_out[h])

                def stats_ns_half(h):
                    c0 = 256 * h  # nsh01 column base for this half
                    # mean column and its broadcast row
                    mean_c = statsp.tile([128, 1], f32, name=f"mean{h}")
                    nc.vector.tensor_scalar(
                        out=mean_c, in0=stats[h][:, 64:65],
                        scalar1=inv_m, scalar2=None, op0=MULT)
                    pmr = scrA[0:1, 256:384]
                    nc.tensor.transpose(pmr, mean_c, id_f)
                    mrow = statsp.tile([1, 128], f32, name=f"mrow{h}")
                    nc.vector.tensor_copy(mrow, pmr)
                    pouter = scrA[:, 0:128]
                    nc.tensor.matmul(pouter, mrow, mrow,
                                     start=True, stop=True)

                    # Sigma in packed columns [g_top; g_bot] [128, 64]
                    sig = nss.tile([128, 64], f32, name=f"sig{h}")
                    nc.vector.tensor_scalar(
                        out=sig, in0=stats[h][:, 0:64],
                        scalar1=inv_m, scalar2=None, op0=MULT)
                    for k, sl in enumerate(blksl):
                        nc.vector.tensor_tensor(
                            out=sig[sl, :], in0=sig[sl, :],
                            in1=pouter[sl, sl], op=SUB)
                    nc.vector.tensor_tensor(
                        out=sig, in0=sig, in1=epsI2, op=ADD)
                    dtmp = nss.tile([128, 64], f32, tag="dtmp", name="dtmp")
                    nc.vector.tensor_tensor(out=dtmp, in0=sig,
                                            in1=eye2, op=MULT)
                    tcol = statsp.tile([128, 1], f32, name=f"tcol{h}")
                    nc.vector.reduce_sum(out=tcol, in_=dtmp, axis=AX)
                    nc.tensor.matmul(scrA[0:1, 384:386], tcol, gmask,
                                     start=True, stop=True)
                    traces = statsp.tile([1, 2], f32, name=f"traces{h}")
                    nc.vector.tensor_copy(traces, scrA[0:1, 384:386])
                    rtr = statsp.tile([1, 2], f32, name=f"rtr{h}")
                    nc.vector.reciprocal(rtr, traces)
                    srtr = statsp.tile([1, 2], f32, name=f"srtr{h}")
                    nc.scalar.sqrt(srtr, rtr)
                    nc.tensor.matmul(scrA[:, 386:388], ones_row, rtr,
                                     start=True, stop=True)
                    nc.tensor.matmul(scrA[:, 388:390], ones_row, srtr,
                                     start=True, stop=True)
                    bc = statsp.tile([128, 4], f32, name=f"bc{h}")
                    nc.vector.tensor_copy(bc, scrA[:, 386:390])
                    sel = statsp.tile([128, 2], f32, tag="sel", name="sel")
                    nc.vector.tensor_tensor(out=sel, in0=bc[:, 0:2],
                                            in1=gmask, op=MULT)
                    rcol = statsp.tile([128, 1], f32, name=f"rcol{h}")
                    nc.vector.reduce_sum(out=rcol, in_=sel, axis=AX)
                    sel2 = statsp.tile([128, 2], f32, tag="sel", name="sel2")
                    nc.vector.tensor_tensor(out=sel2, in0=bc[:, 2:4],
                                            in1=gmask, op=MULT)
                    scol = statsp.tile([128, 1], f32, name=f"scol{h}")
                    nc.vector.reduce_sum(out=scol, in_=sel2, axis=AX)

                    # SigN block-diag
                    sigN = nss.tile([128, 128], f32, name=f"sigN{h}")
                    for k, sl in enumerate(blksl):
                        nc.vector.tensor_scalar(
                            out=sigN[sl, sl], in0=sig[sl, :],
                            scalar1=rcol[sl, :], scalar2=None, op0=MULT)

                    # Newton-Schulz: A = P*P and B = SigN*P, then C = A*B
                    P = id_f
                    for t in range(T_ITERS):
                        pn = nss.tile([128, 128], f32, tag=f"P{h}",
                                      name=f"P{h}_{t}")
                        for sl in blksl:
                            nc.vector.tensor_scalar(
                                out=pn[sl, sl], in0=P[sl, sl],
                                scalar1=1.5, scalar2=None, op0=MULT)
                        for k, sl in enumerate(blksl):
                            nc.tensor.matmul(
                                nsh01[sl, c0:c0 + 64], P[sl, sl], P[sl, sl],
                                start=True, stop=True,
                                tile_position=(64 * k, 64 * k))
                        for k, sl in enumerate(blksl):
                            nc.tensor.matmul(
                                nsh01[sl, c0 + 64:c0 + 128], sigN[sl, sl],
                                P[sl, sl], start=True, stop=True,
                                tile_position=(64 * k, 64 * k))
                        asb = nss.tile([128, 64], f32, tag=f"asb{h}",
                                       name="asb")
                        nc.vector.tensor_copy(asb, nsh01[:, c0:c0 + 64])
                        bsb = nss.tile([128, 64], f32, tag=f"bsb{h}",
                                       name="bsb")
                        nc.scalar.copy(bsb, nsh01[:, c0 + 64:c0 + 128])
                        for k, sl in enumerate(blksl):
                            nc.tensor.matmul(
                                nsh01[sl, c0 + 128:c0 + 192], asb[sl, :],
                                bsb[sl, :], start=True, stop=True,
                                tile_position=(64 * k, 64 * k))
                        cs = nss.tile([128, 64], f32, tag=f"cs{h}",
                                      name="cs")
                        nc.scalar.mul(cs, nsh01[:, c0 + 128:c0 + 192], -0.5)
                        for sl in blksl:
                            nc.vector.tensor_tensor(
                                out=pn[sl, sl], in0=pn[sl, sl],
                                in1=cs[sl, :], op=ADD)
                        P = pn

                    # block-diag bf16 lhsT (weight * srtr * P) + offset
                    nc.vector.tensor_copy(bd[h], zeros_bf)
                    for k, sl in enumerate(blksl):
                        nc.vector.tensor_scalar(
                            out=bd[h][sl, sl], in0=P[sl, sl],
                            scalar1=scol[sl, :], scalar2=None, op0=MULT)
                        nc.vector.tensor_tensor(
                            out=bd[h][sl, sl], in0=bd[h][sl, sl],
                            in1=wbc[sl, 128 * h + sl.start:
                                    128 * h + sl.stop], op=MULT)
                    mb = statsp.tile([128, 1], bf16, name=f"meanbf{h}")
                    nc.vector.tensor_copy(mb, mean_c)
                    pwm = scrA[:, 390 + h:391 + h]
                    nc.tensor.matmul(pwm, bd[h], mb, start=True, stop=True)
                    nc.vector.tensor_tensor(
                        out=offs[h], in0=bcol[:, h:h + 1],
                        in1=pwm, op=SUB)

                def apply_half(h):
                    hs = slice(h * 128, (h + 1) * 128)
                    base = h * HW
                    for b in range(BS):
                        aot = apo.tile([128, HW], f32, tag="aot", name="aot")
                        for ci in range(HW // APPLY_N):
                            nsl = slice(base + ci * APPLY_N,
                                        base + (ci + 1) * APPLY_N)
                            osl = slice(ci * APPLY_N, (ci + 1) * APPLY_N)
                            pap = app.tile([128, APPLY_N], f32, tag="pap",
                                           name="pap")
                            nc.tensor.matmul(pap, bd[h], xt[b][:, nsl],
                                             start=True, stop=True)
                            if (ci + b) % 2 == 0:
                                nc.vector.tensor_scalar(
                                    out=aot[:, osl], in0=pap,
                                    scalar1=offs[h], scalar2=None, op0=ADD)
                            else:
                                nc.scalar.add(aot[:, osl], pap, offs[h])
                        nc.sync.dma_start(out=Od[b, hs, :], in_=aot)

                # ---- pipelined schedule ----
                phase1_half(0)
                start_allreduce(0)      # AR(h0) flies under h1 phase-1
                phase1_half(1)
                stats_ns_half(0)
                start_allreduce(1)      # AR(h1) flies under h0 apply
                apply_half(0)
                stats_ns_half(1)
                apply_half(1)
    nc.compile()
    return nc


def kernel(X, weight, bias):
    from concourse.bass_utils import run_bass_kernel_spmd

    if "nc" not in _CACHE:
        _CACHE["nc"] = _build_nc()
    nc = _CACHE["nc"]

    X = np.ascontiguousarray(np.asarray(X, dtype=np.float32)).reshape(B, C, HW)
    w = np.ascontiguousarray(np.asarray(weight, dtype=np.float32)).reshape(C)
    bb = np.ascontiguousarray(np.asarray(bias, dtype=np.float32)).reshape(C)
    in_maps = [
        {"X": np.ascontiguousarray(X[i * BS:(i + 1) * BS]),
         "weight": w, "bias": bb}
        for i in range(NCORES)
    ]
    res = run_bass_kernel_spmd(nc, in_maps, core_ids=list(range(NCORES)))
    _CACHE["last_result"] = res
    out = np.concatenate([r["out"] for r in res.results], axis=0)
    return out.reshape(B, C, H, W)


# revision 15
# speedup vs baseline: 1.8995x; 1.1297x over previous
"""IterNorm (iterative whitening normalization) Bass kernel for 8 TRN2 cores.

Shapes (hardcoded): X (64, 256, 56, 56) f32; g=4 groups of d=64 channels;
m = 64*56*56 = 200704; Sigma = eps*I + (1/m) xc xc^T per group; 5
Newton-Schulz iters -> whitening wm; out = (wm @ xc) * weight + bias.

Sharding: data-parallel over batch B (8 b's per core), with the channel
halves (h = channels 0:128 / 128:256, i.e. groups {0,1} / {2,3}) pipelined
so the two 33KB stats all-reduces hide under compute:

  [h0 phase-1: transpose+cov 8 tiles] -> AR(h0) in flight
  [h1 phase-1]                           (covers AR(h0) latency)
  [h0 stats+NewtonSchulz]  -> AR(h1) in flight right after h1 cov stop
  [h0 apply + output DMA]                (covers AR(h1) latency)
  [h1 stats+NS] [h1 apply + output DMA]

Phase 1 tricks: cast-DMA f32->bf16 loads (all tiles resident in SBUF); PE
transposes into bf16 PSUM tiles with persistent ones-columns; cov matmuls
use rhs [chunk | ones] so channel sums fall out of col 128 for free; DVE
does the PSUM->SBUF copies (ACT faults on bf16 PSUM reads).
Newton-Schulz packs each half's 2 groups into diagonal 64x64 quadrants
(tile_position), computing A=P^2 and B=SigN*P back-to-back then C=A*B to
halve the serial chain. Apply is a single block-diagonal bf16 128-contract
matmul per 448-col chunk (weight*sqrt(rTr)*P folded into lhsT, bias-W@mean
folded into a per-partition offset added on alternating DVE/ACT).
"""

import numpy as np

B, C, H, W = 64, 256, 56, 56
HW = H * W               # 3136
G, D = 4, 64             # groups, channels/group
NCORES = 8
BS = B // NCORES         # 8 batches per core
M = B * HW               # 200704 (full reduction length)
EPS = 1e-5
T_ITERS = 5

NCH = 128                # transpose chunk width (hw)
NCHUNK = 25              # 24 full chunks + 1 tail of 64
TAIL = HW - 24 * NCH     # 64
APPLY_N = 448            # apply matmul free dim; 7 * 448 = 3136

_CACHE = {}


def _build_nc(single_core_sim=False):
    import concourse.bacc as bacc
    import concourse.tile as tile
    from concourse import mybir
    import ml_dtypes

    f32 = mybir.dt.float32
    bf16 = mybir.dt.bfloat16
    AX = mybir.AxisListType.X
    ADD = mybir.AluOpType.add
    SUB = mybir.AluOpType.subtract
    MULT = mybir.AluOpType.mult

    nc = bacc.Bacc(
        "TRN2",
        target_bir_lowering=False,
        debug=False,
        enable_asserts=False,
        num_devices=1 if single_core_sim else NCORES,
    )
    Xd = nc.dram_tensor("X", [BS, C, HW], f32, kind="ExternalInput").ap()
    Wd = nc.dram_tensor("weight", [C], f32, kind="ExternalInput").ap()
    Bd = nc.dram_tensor("bias", [C], f32, kind="ExternalInput").ap()
    Od = nc.dram_tensor("out", [BS, C, HW], f32, kind="ExternalOutput").ap()

    widths = [NCH] * 24 + [TAIL]
    cgroups = [tuple(range(3 * i, 3 * i + 3)) for i in range(8)] + [(24,)]
    inv_m = 1.0 / float(M)
    blksl = [slice(0, 64), slice(64, 128)]

    id_np = np.eye(128, dtype=np.float32)
    eye2_np = np.vstack([np.eye(64, dtype=np.float32)] * 2)      # [128, 64]
    gmask_np = np.zeros((128, 2), dtype=np.float32)
    gmask_np[0:64, 0] = 1.0
    gmask_np[64:128, 1] = 1.0

    with tile.TileContext(nc) as tc:
        with (
            tc.tile_pool(name="consts", bufs=1) as consts,
            tc.tile_pool(name="resx", bufs=1) as resx,
            tc.tile_pool(name="stp", bufs=1) as stp,
            tc.tile_pool(name="statsp", bufs=1) as statsp,
            tc.tile_pool(name="nss", bufs=2) as nss,
            tc.tile_pool(name="apo", bufs=3) as apo,
            tc.tile_pool(name="dram", bufs=1, space="DRAM") as dram,
        ):
            # ---- constants ----
            id_bf = consts.tile([128, 128], bf16)
            nc.sync.dma_start(out=id_bf, in_=nc.inline_tensor(
                id_np.astype(ml_dtypes.bfloat16), name="id_bf").ap())
            id_f = consts.tile([128, 128], f32)
            nc.sync.dma_start(out=id_f, in_=nc.inline_tensor(
                id_np, name="id_f").ap())
            zeros_bf = consts.tile([128, 128], bf16)
            nc.sync.dma_start(out=zeros_bf, in_=nc.inline_tensor(
                np.zeros((128, 128), dtype=ml_dtypes.bfloat16),
                name="zeros_bf").ap())
            eye2 = consts.tile([128, 64], f32)
            nc.sync.dma_start(out=eye2, in_=nc.inline_tensor(
                eye2_np, name="eye2").ap())
            epsI2 = consts.tile([128, 64], f32)
            nc.sync.dma_start(out=epsI2, in_=nc.inline_tensor(
                EPS * eye2_np, name="epsI2").ap())
            gmask = consts.tile([128, 2], f32)
            nc.sync.dma_start(out=gmask, in_=nc.inline_tensor(
                gmask_np, name="gmask").ap())
            ones_row = consts.tile([1, 128], f32)
            nc.sync.dma_start(out=ones_row, in_=nc.inline_tensor(
                np.ones((1, 128), dtype=np.float32), name="ones_row").ap())
            wrow = consts.tile([1, C], f32)
            nc.sync.dma_start(out=wrow, in_=Wd[None, :])
            bcol = consts.tile([128, 2], f32)
            nc.sync.dma_start(out=bcol[:, 0:1], in_=Bd[0:128][:, None])
            nc.sync.dma_start(out=bcol[:, 1:2], in_=Bd[128:256][:, None])

            # ---- resident input tiles: cast-DMA f32 -> bf16, h-major ----
            xt = [resx.tile([128, 2 * HW], bf16, name=f"xt{b}")
                  for b in range(BS)]
            for h in range(2):
                for b in range(BS):
                    nc.gpsimd.dma_start(
                        out=xt[b][:, h * HW:(h + 1) * HW],
                        in_=Xd[b, 128 * h:128 * (h + 1), :])

            st_bufs = [stp.tile([128, 390], bf16, name=f"stb{i}")
                       for i in range(4)]
            cc_in = [statsp.tile([128, 66], bf16, name=f"cc_in{h}")
                     for h in range(2)]
            stats = [statsp.tile([128, 66], bf16, name=f"stats{h}")
                     for h in range(2)]
            bounce_in = [dram.tile([128, 66], bf16, name=f"bin{h}")
                         for h in range(2)]
            bounce_out = [dram.tile([128, 66], bf16, name=f"bout{h}")
                          for h in range(2)]
            bd = [nss.tile([128, 128], bf16, name=f"bd{h}") for h in range(2)]
            offs = [statsp.tile([128, 1], f32, name=f"offs{h}")
                    for h in range(2)]

            ns_state = {}
            with tc.tile_pool(name="p1ps", bufs=1, space="PSUM") as p1ps:
                pt_bufs = [p1ps.tile([128, 512], bf16, name=f"ptb{i}")
                           for i in range(2)]
                cov = [p1ps.tile([128, 512], f32, name=f"cov{h}")
                       for h in range(2)]

                for ptb in pt_bufs:
                    for oc in (128, 258, 388):
                        nc.vector.memset(
                            ptb[:, oc:oc + 2].bitcast(mybir.dt.uint32),
                            0x3F803F80)

                gctr = 0

                def phase1_half(h):
                    nonlocal gctr
                    base = h * HW
                    for b in range(BS):
                        for chunks in cgroups:
                            ptb = pt_bufs[gctr % 2]
                            stb = st_bufs[gctr % 4]
                            ncols = len(chunks) * 130
                            for j, k in enumerate(chunks):
                                kw = widths[k]
                                nc.tensor.transpose(
                                    ptb[0:kw, j * 130:j * 130 + 128],
                                    xt[b][:, base + k * NCH:
                                          base + k * NCH + kw],
                                    id_bf,
                                )
                            # ACT faults on bf16 PSUM reads; DVE-only copy
                            nc.vector.tensor_copy(
                                stb[0:128, 0:ncols], ptb[0:128, 0:ncols])
                            for j, k in enumerate(chunks):
                                kw = widths[k]
                                nc.tensor.matmul(
                                    cov[h][:, 0:129],
                                    stb[0:kw, j * 130:j * 130 + 128],
                                    stb[0:kw, j * 130:j * 130 + 129],
                                    start=(b == 0 and k == 0),
                                    stop=(b == BS - 1 and k == NCHUNK - 1),
                                )
                            gctr += 1

                def start_allreduce(h):
                    nc.vector.tensor_copy(cc_in[h][0:64, 0:64],
                                          cov[h][0:64, 0:64])
                    nc.vector.tensor_copy(cc_in[h][64:128, 0:64],
                                          cov[h][64:128, 64:128])
                    nc.vector.tensor_copy(cc_in[h][:, 64:65],
                                          cov[h][:, 128:129])
                    nc.sync.dma_start(out=bounce_in[h], in_=cc_in[h])
                    if single_core_sim:
                        nc.sync.dma_start(out=bounce_out[h],
                                          in_=bounce_in[h])
                    else:
                        nc.gpsimd.collective_compute(
                            "AllReduce",
                            mybir.AluOpType.add,
                            replica_groups=[list(range(NCORES))],
                            ins=[bounce_in[h].opt()],
                            outs=[bounce_out[h].opt()],
                        )
                    nc.sync.dma_start(out=stats[h], in_=bounce_out[h])

                def stats_ns_half(h):
                    scrA = ns_state["scrA"]
                    nsh01 = ns_state["nsh01"]
                    wbc = ns_state["wbc"]
                    c0 = 256 * h  # nsh01 column base for this half
                    # mean column and its broadcast row
                    mean_c = statsp.tile([128, 1], f32, name=f"mean{h}")
                    nc.vector.tensor_scalar(
                        out=mean_c, in0=stats[h][:, 64:65],
                        scalar1=inv_m, scalar2=None, op0=MULT)
                    pmr = scrA[0:1, 256:384]
                    nc.tensor.transpose(pmr, mean_c, id_f)
                    mrow = statsp.tile([1, 128], f32, name=f"mrow{h}")
                    nc.vector.tensor_copy(mrow, pmr)
                    pouter = scrA[:, 0:128]
                    nc.tensor.matmul(pouter, mrow, mrow,
                                     start=True, stop=True)

                    # Sigma in packed columns [g_top; g_bot] [128, 64]
                    sig = nss.tile([128, 64], f32, name=f"sig{h}")
                    nc.vector.tensor_scalar(
                        out=sig, in0=stats[h][:, 0:64],
                        scalar1=inv_m, scalar2=None, op0=MULT)
                    for k, sl in enumerate(blksl):
                        nc.vector.tensor_tensor(
                            out=sig[sl, :], in0=sig[sl, :],
                            in1=pouter[sl, sl], op=SUB)
                    nc.vector.tensor_tensor(
                        out=sig, in0=sig, in1=epsI2, op=ADD)
                    dtmp = nss.tile([128, 64], f32, tag="dtmp", name="dtmp")
                    nc.vector.tensor_tensor(out=dtmp, in0=sig,
                                            in1=eye2, op=MULT)
                    tcol = statsp.tile([128, 1], f32, name=f"tcol{h}")
                    nc.vector.reduce_sum(out=tcol, in_=dtmp, axis=AX)
                    nc.tensor.matmul(scrA[0:1, 384:386], tcol, gmask,
                                     start=True, stop=True)
                    traces = statsp.tile([1, 2], f32, name=f"traces{h}")
                    nc.vector.tensor_copy(traces, scrA[0:1, 384:386])
                    rtr = statsp.tile([1, 2], f32, name=f"rtr{h}")
                    nc.vector.reciprocal(rtr, traces)
                    srtr = statsp.tile([1, 2], f32, name=f"srtr{h}")
                    nc.scalar.sqrt(srtr, rtr)
                    nc.tensor.matmul(scrA[:, 386:388], ones_row, rtr,
                                     start=True, stop=True)
                    nc.tensor.matmul(scrA[:, 388:390], ones_row, srtr,
                                     start=True, stop=True)
                    bc = statsp.tile([128, 4], f32, name=f"bc{h}")
                    nc.vector.tensor_copy(bc, scrA[:, 386:390])
                    sel = statsp.tile([128, 2], f32, tag="sel", name="sel")
                    nc.vector.tensor_tensor(out=sel, in0=bc[:, 0:2],
                                            in1=gmask, op=MULT)
                    rcol = statsp.tile([128, 1], f32, name=f"rcol{h}")
                    nc.vector.reduce_sum(out=rcol, in_=sel, axis=AX)
                    sel2 = statsp.tile([128, 2], f32, tag="sel", name="sel2")
                    nc.vector.tensor_tensor(out=sel2, in0=bc[:, 2:4],
                                            in1=gmask, op=MULT)
                    scol = statsp.tile([128, 1], f32, name=f"scol{h}")
                    nc.vector.reduce_sum(out=scol, in_=sel2, axis=AX)

                    # SigN block-diag
                    sigN = nss.tile([128, 128], f32, name=f"sigN{h}")
                    for k, sl in enumerate(blksl):
                        nc.vector.tensor_scalar(
                            out=sigN[sl, sl], in0=sig[sl, :],
                            scalar1=rcol[sl, :], scalar2=None, op0=MULT)

                    # Newton-Schulz: A = P*P and B = SigN*P, then C = A*B
                    P = id_f
                    for t in range(T_ITERS):
                        pn = nss.tile([128, 128], f32, tag=f"P{h}",
                                      name=f"P{h}_{t}")
                        for sl in blksl:
                            nc.vector.tensor_scalar(
                                out=pn[sl, sl], in0=P[sl, sl],
                                scalar1=1.5, scalar2=None, op0=MULT)
                        for k, sl in enumerate(blksl):
                            nc.tensor.matmul(
                                nsh01[sl, c0:c0 + 64], P[sl, sl], P[sl, sl],
                                start=True, stop=True,
                                tile_position=(64 * k, 64 * k))
                        for k, sl in enumerate(blksl):
                            nc.tensor.matmul(
                                nsh01[sl, c0 + 64:c0 + 128], sigN[sl, sl],
                                P[sl, sl], start=True, stop=True,
                                tile_position=(64 * k, 64 * k))
                        asb = nss.tile([128, 64], f32, tag=f"asb{h}",
                                       name="asb")
                        nc.vector.tensor_copy(asb, nsh01[:, c0:c0 + 64])
                        bsb = nss.tile([128, 64], f32, tag=f"bsb{h}",
                                       name="bsb")
                        nc.scalar.copy(bsb, nsh01[:, c0 + 64:c0 + 128])
                        for k, sl in enumerate(blksl):
                            nc.tensor.matmul(
                                nsh01[sl, c0 + 128:c0 + 192], asb[sl, :],
                                bsb[sl, :], start=True, stop=True,
                                tile_position=(64 * k, 64 * k))
                        cs = nss.tile([128, 64], f32, tag=f"cs{h}",
                                      name="cs")
                        nc.scalar.mul(cs, nsh01[:, c0 + 128:c0 + 192], -0.5)
                        for sl in blksl:
                            nc.vector.tensor_tensor(
                                out=pn[sl, sl], in0=pn[sl, sl],
                                in1=cs[sl, :], op=ADD)
                        P = pn

                    # block-diag bf16 lhsT (weight * srtr * P) + offset
                    nc.vector.tensor_copy(bd[h], zeros_bf)
                    for k, sl in enumerate(blksl):
                        nc.vector.tensor_scalar(
                            out=bd[h][sl, sl], in0=P[sl, sl],
                            scalar1=scol[sl, :], scalar2=None, op0=MULT)
                        nc.vector.tensor_tensor(
                            out=bd[h][sl, sl], in0=bd[h][sl, sl],
                            in1=wbc[sl, 128 * h + sl.start:
                                    128 * h + sl.stop], op=MULT)
                    mb = statsp.tile([128, 1], bf16, name=f"meanbf{h}")
                    nc.vector.tensor_copy(mb, mean_c)
                    pwm = scrA[:, 390 + h:391 + h]
                    nc.tensor.matmul(pwm, bd[h], mb, start=True, stop=True)
                    nc.vector.tensor_tensor(
                        out=offs[h], in0=bcol[:, h:h + 1],
                        in1=pwm, op=SUB)

                def apply_half(h, bs):
                    app = ns_state["app"]
                    hs = slice(h * 128, (h + 1) * 128)
                    base = h * HW
                    for b in bs:
                        aot = apo.tile([128, HW], f32, tag="aot", name="aot")
                        for ci in range(HW // APPLY_N):
                            nsl = slice(base + ci * APPLY_N,
                                        base + (ci + 1) * APPLY_N)
                            osl = slice(ci * APPLY_N, (ci + 1) * APPLY_N)
                            pap = app.tile([128, APPLY_N], f32, tag="pap",
                                           name="pap")
                            nc.tensor.matmul(pap, bd[h], xt[b][:, nsl],
                                             start=True, stop=True)
                            if (ci + b) % 2 == 0:
                                nc.vector.tensor_scalar(
                                    out=aot[:, osl], in0=pap,
                                    scalar1=offs[h], scalar2=None, op0=ADD)
                            else:
                                nc.scalar.add(aot[:, osl], pap, offs[h])
                        nc.sync.dma_start(out=Od[b, hs, :], in_=aot)

                # ---- phase 1 + both AR issues (p1ps banks then freed) --
                phase1_half(0)
                start_allreduce(0)      # AR(h0) flies under h1 phase-1
                phase1_half(1)
                start_allreduce(1)      # AR(h1) flies under h0 apply

            with (
                tc.tile_pool(name="nsps", bufs=1, space="PSUM") as nsps,
                tc.tile_pool(name="app", bufs=4, space="PSUM") as app,
            ):
                scrA = nsps.tile([128, 512], f32, name="scrA")
                nsh01 = nsps.tile([128, 512], f32, name="nsh01")
                ns_state["scrA"] = scrA
                ns_state["nsh01"] = nsh01
                ns_state["app"] = app
                # weight broadcast [128, 256] (h-independent)
                nc.tensor.matmul(scrA[:, 0:256], ones_row, wrow,
                                 start=True, stop=True)
                wbc = nss.tile([128, 256], f32, name="wbc")
                nc.vector.tensor_copy(wbc, scrA[:, 0:256])
                ns_state["wbc"] = wbc

                stats_ns_half(0)
                apply_half(0, range(7))
                stats_ns_half(1)        # AR(h1) landed during h0 apply
                apply_half(0, [7])
                apply_half(1, range(BS))
    nc.compile()
    return nc


def kernel(X, weight, bias):
    from concourse.bass_utils import run_bass_kernel_spmd

    if "nc" not in _CACHE:
        _CACHE["nc"] = _build_nc()
    nc = _CACHE["nc"]

    X = np.ascontiguousarray(np.asarray(X, dtype=np.float32)).reshape(B, C, HW)
    w = np.ascontiguousarray(np.asarray(weight, dtype=np.float32)).reshape(C)
    bb = np.ascontiguousarray(np.asarray(bias, dtype=np.float32)).reshape(C)
    in_maps = [
        {"X": np.ascontiguousarray(X[i * BS:(i + 1) * BS]),
         "weight": w, "bias": bb}
        for i in range(NCORES)
    ]
    res = run_bass_kernel_spmd(nc, in_maps, core_ids=list(range(NCORES)))
    _CACHE["last_result"] = res
    out = np.concatenate([r["out"] for r in res.results], axis=0)
    return out.reshape(B, C, H, W)
